# revision 12
# baseline (speedup 1.0000x reference)
"""Single-head attention Trainium2 kernel (batch=8 data-parallel over 8 cores).

Reference (per batch element): out = softmax((x Wq)(x Wk)^T / 8) (x Wv).

Strategy: fp8-e4m3 residual (hi+lo) arithmetic on the PE at DoubleRow rate
(0.5 cycles/row, 256-deep contraction per instruction), with all first-order
quantization error cancelled by 3-pass matmuls (hi*hi + lo*hi + hi*lo).
Measured end-to-end absmax_rel ~3e-3 (gate 2e-2).

Algebraic restructure: scores = x M x^T with M = Wq Wk^T precomputed on the
host (exact f32), so the device never materializes Q or K:
    G'^T = M^T-contraction vs x^T     [d', s]
    scoresT = x^T-contraction vs G'^T [ks, q]
    expT = exp(scores/8 - 5.5)        (global exp bias cancels in softmax)
    A^T  = x-contraction vs expT      [d, q]   (= (attn @ x)^T unnormalized)
    out  = A-contraction vs Wv, * 1/rowsum
Host also pre-splits M and Wv into hi/lo e4m3 pair-tile layouts (pure weight
preprocessing; x-dependent work all happens on device).

Power-of-2 scales keep every rescale exact: sx=16, sM=2048, sG=64, sWv=1024,
sA=1/4; hi and lo share one scale so all 3 residual passes accumulate in a
single PSUM group.
"""

import sys

sys.path.insert(0, "/opt/trn_rl_repo")

from contextlib import ExitStack

import numpy as np
import ml_dtypes

import concourse.bass as bass
import concourse.mybir as mybir
from concourse import bacc
from concourse.tile import TileContext
from concourse.masks import make_identity

F32 = mybir.dt.float32
F32R = mybir.dt.float32r
E4 = mybir.dt.float8e4
U8 = mybir.dt.uint8
DR = mybir.MatmulPerfMode.DoubleRow
EXP = mybir.ActivationFunctionType.Exp
COPY = mybir.ActivationFunctionType.Copy
MULT = mybir.AluOpType.mult
SUBTRACT = mybir.AluOpType.subtract

E4NP = ml_dtypes.float8_e4m3

S, D, O = 2048, 1024, 1024
SX = 16.0          # x scale
SM = 2048.0        # M scale
SG = 64.0          # G' scale
SV = 1024.0        # Wv scale
SA = 0.25          # A scale
BIAS = -5.5        # exp bias (cancels in softmax), keeps e4m3 in range
C_G = SG / (SX * SM)        # 2^-9  : psum(G'*sx*sM) -> G'*sG
C_E = (1.0 / 8.0) / (SX * SG)  # 2^-13: psum(scores*sx*sG) -> scores/8
C_A = SA / SX               # 2^-6  : psum(A*sx) -> A*sA
C_O = 1.0 / (SA * SV)       # 2^-8  : psum(out*sA*sV) -> out (pre-rowsum)


def build_attn(reps=1):
    """Bass module for one core: x[S,D] f32, mprep/vprep fp8 -> out[S,O] f32."""
    KC = S // 128      # 16 ks-chunks
    KP = KC // 2       # 8 ks-pairs
    DP = D // 256      # 4 d-pairs
    NSB = S // 512     # 4 s-blocks (also the q-blocks)

    nc = bacc.Bacc("TRN2", target_bir_lowering=False, debug=False)
    x_in = nc.dram_tensor("x", [S, D], F32, kind="ExternalInput")
    m_in = nc.dram_tensor("mprep", [2, DP, 128, 2, D], U8, kind="ExternalInput")
    v_in = nc.dram_tensor("vprep", [2, DP, 128, 2, O], U8, kind="ExternalInput")
    out_d = nc.dram_tensor("out", [S, O], F32, kind="ExternalOutput")

    with TileContext(nc) as tc:
      for _rep in range(reps):
        top = ExitStack()
        const_pool = top.enter_context(tc.tile_pool(name="constp", bufs=4))
        w_pool = top.enter_context(tc.tile_pool(name="wp", bufs=4 * DP))
        xr_pool = top.enter_context(tc.tile_pool(name="xrp", bufs=2 * KP))
        xt_pool = top.enter_context(tc.tile_pool(name="xtp", bufs=2 * DP * NSB))
        gt_pool = top.enter_context(tc.tile_pool(name="gtp", bufs=2 * DP * NSB))

        ident_f = const_pool.tile([128, 128], F32, tag="identf")
        make_identity(nc, ident_f)
        ident_r = const_pool.tile([128, 128], F32R, tag="identr")
        nc.vector.tensor_copy(out=ident_r, in_=ident_f)
        ones8 = const_pool.tile([128, 2, 1], E4, tag="ones8")
        nc.gpsimd.memset(ones8, 1.0)
        bias_t = const_pool.tile([128, 1], F32, tag="biast")
        nc.gpsimd.memset(bias_t, BIAS)

        # -------- weight tiles (DMAs deferred into the s-block loop so the
        # first x rows win the queue; vprep is only needed in phase 4) ----
        mst = [[None] * DP for _ in range(2)]   # [hl][dp] -> [128, 2, D]
        wvt = [[None] * DP for _ in range(2)]
        for hl in range(2):
            for dp in range(DP):
                mst[hl][dp] = w_pool.tile([128, 2, D], E4, tag="mst",
                                          bufs=2 * DP, name=f"mst_{hl}_{dp}")
                wvt[hl][dp] = w_pool.tile([128, 2, O], E4, tag="wvt",
                                          bufs=2 * DP, name=f"wvt_{hl}_{dp}")

        # -------- phases 1-3 interleaved per s-block --------
        # load + row-split (ACT hi / DVE lo), f32r transposes with split on
        # PSUM eviction, then G'^T DoubleRow matmuls with split on eviction.
        xr8 = [[None] * KP for _ in range(2)]   # [hl][kp] -> [128, 2, D]
        for hl in range(2):
            for kp in range(KP):
                xr8[hl][kp] = xr_pool.tile([128, 2, D], E4, tag="xr",
                                           bufs=2 * KP, name=f"xr_{hl}_{kp}")
        with ExitStack() as ph1:
            xn_pool = ph1.enter_context(tc.tile_pool(name="xnp", bufs=8))
            xt8 = [[[None] * NSB for _ in range(DP)] for _ in range(2)]
            gt8 = [[[None] * NSB for _ in range(DP)] for _ in range(2)]
            pst = ph1.enter_context(tc.tile_pool(name="pst", bufs=3, space="PSUM"))
            psg = ph1.enter_context(tc.tile_pool(name="psg", bufs=3, space="PSUM"))
            def g_stage(sb):
                for dpc in range(D // 128):
                    pg = psg.tile([128, 512], F32, tag="pg", bufs=3)
                    n = 0
                    for (hm, hx) in ((0, 0), (0, 1), (1, 0)):
                        for dp in range(DP):
                            nc.tensor.matmul(
                                pg,
                                mst[hm][dp][:, :, dpc * 128:(dpc + 1) * 128],
                                xt8[hx][dp][sb],
                                start=(n == 0), stop=(n == 3 * DP - 1),
                                perf_mode=DR)
                            n += 1
                    if gt8[0][dpc // 2][sb] is None:
                        gt8[0][dpc // 2][sb] = gt_pool.tile(
                            [128, 2, 512], E4, tag="gt",
                            bufs=2 * DP * NSB, name=f"gt_0_{dpc}_{sb}")
                        gt8[1][dpc // 2][sb] = gt_pool.tile(
                            [128, 2, 512], E4, tag="gt",
                            bufs=2 * DP * NSB, name=f"gt_1_{dpc}_{sb}")
                    dh = gt8[0][dpc // 2][sb]
                    dl = gt8[1][dpc // 2][sb]
                    nc.scalar.activation(out=dh[:, dpc % 2, :], in_=pg,
                                         func=COPY, scale=C_G)
                    nc.vector.scalar_tensor_tensor(
                        out=dl[:, dpc % 2, :], in0=pg, scalar=C_G,
                        in1=dh[:, dpc % 2, :], op0=MULT, op1=SUBTRACT)

            for sb in range(NSB):
                xns = []
                for ss in range(4):
                    kc = sb * 4 + ss
                    xn = xn_pool.tile([128, D], F32R, tag="xn", bufs=8)
                    dma_eng = nc.sync if kc % 2 == 0 else nc.gpsimd
                    for colh in range(2):
                        dma_eng.dma_start(
                            out=xn[:, colh * 512:(colh + 1) * 512],
                            in_=x_in[kc * 128:(kc + 1) * 128,
                                     colh * 512:(colh + 1) * 512].bitcast(F32R))
                    xns.append(xn)
                    kp, h = kc // 2, kc % 2
                    nc.scalar.activation(out=xr8[0][kp][:, h, :],
                                         in_=xn.bitcast(F32), func=COPY, scale=SX)
                    nc.vector.scalar_tensor_tensor(
                        out=xr8[1][kp][:, h, :], in0=xn.bitcast(F32), scalar=SX,
                        in1=xr8[0][kp][:, h, :], op0=MULT, op1=SUBTRACT)
                if sb == 1:
                    for hl in range(2):
                        for dp in range(DP):
                            eng = nc.sync if dp % 2 == 0 else nc.gpsimd
                            eng.dma_start(out=mst[hl][dp],
                                          in_=m_in[hl, dp].bitcast(E4))
                elif sb == 3:
                    for hl in range(2):
                        for dp in range(DP):
                            eng = nc.sync if dp % 2 == 0 else nc.gpsimd
                            eng.dma_start(out=wvt[hl][dp],
                                          in_=v_in[hl, dp].bitcast(E4))
                # transposes: x[sb-block rows] -> xT, split hi/lo on eviction
                for dc in range(D // 128):
                    pt = pst.tile([128, 512], F32R, tag="pt", bufs=3)
                    for ss in range(4):
                        nc.tensor.transpose(
                            pt[:, ss * 128:(ss + 1) * 128],
                            xns[ss][:, dc * 128:(dc + 1) * 128],
                            ident_r,
                        )
                    if xt8[0][dc // 2][sb] is None:
                        for hl in range(2):
                            xt8[hl][dc // 2][sb] = xt_pool.tile(
                                [128, 2, 512], E4, tag="xt",
                                bufs=2 * DP * NSB, name=f"xt_{hl}_{dc}_{sb}")
                    ptf = pt.bitcast(F32)
                    dh = xt8[0][dc // 2][sb]
                    dl = xt8[1][dc // 2][sb]
                    nc.scalar.activation(out=dh[:, dc % 2, :], in_=ptf,
                                         func=COPY, scale=SX)
                    nc.vector.scalar_tensor_tensor(
                        out=dl[:, dc % 2, :], in0=ptf, scalar=SX,
                        in1=dh[:, dc % 2, :], op0=MULT, op1=SUBTRACT)
                # G' for the previous s-block overlaps this block's
                # transpose evictions (PE never waits on ACT/DVE)
                if sb >= 1:
                    g_stage(sb - 1)
            g_stage(NSB - 1)

        # -------- phase 4: attention per q-block (qb = s-block of 512) ----
        with ExitStack() as ph4:
            e_pool = ph4.enter_context(tc.tile_pool(name="ep", bufs=4 * KP))
            e32_pool = ph4.enter_context(tc.tile_pool(name="e32p", bufs=4))
            a_pool = ph4.enter_context(tc.tile_pool(name="ap", bufs=4 * DP))
            small_pool = ph4.enter_context(tc.tile_pool(name="smallp", bufs=16))
            outs_pool = ph4.enter_context(tc.tile_pool(name="outsp", bufs=3))
            pcs = ph4.enter_context(tc.tile_pool(name="pcs", bufs=3, space="PSUM"))
            pca = ph4.enter_context(tc.tile_pool(name="pca", bufs=2, space="PSUM"))
            pco = ph4.enter_context(tc.tile_pool(name="pco", bufs=2, space="PSUM"))
            pcr = ph4.enter_context(tc.tile_pool(name="pcr", bufs=1, space="PSUM"))

            def scores_stage(qb):
                """scoresT + exp splits for q-block qb -> e8 tiles."""
                e8 = [[None] * KP for _ in range(2)]
                for kc in range(KC):
                    ps = pcs.tile([128, 512], F32, tag="ps", bufs=3)
                    sbk, ss = kc // 4, kc % 4
                    n = 0
                    for (ha, hb) in ((0, 0), (0, 1), (1, 0)):
                        for dp in range(DP):
                            nc.tensor.matmul(
                                ps,
                                xt8[ha][dp][sbk][:, :, ss * 128:(ss + 1) * 128],
                                gt8[hb][dp][qb],
                                start=(n == 0), stop=(n == 3 * DP - 1),
                                perf_mode=DR)
                            n += 1
                    e32 = e32_pool.tile([128, 512], F32, tag="e32", bufs=4)
                    nc.scalar.activation(out=e32, in_=ps, func=EXP,
                                         bias=bias_t, scale=C_E)
                    kp, h = kc // 2, kc % 2
                    if h == 0:
                        e8[0][kp] = e_pool.tile([128, 2, 512], E4, tag="e8",
                                                bufs=4 * KP,
                                                name=f"e8h_{qb}_{kp}")
                        e8[1][kp] = e_pool.tile([128, 2, 512], E4, tag="e8",
                                                bufs=4 * KP,
                                                name=f"e8l_{qb}_{kp}")
                    nc.gpsimd.tensor_copy(out=e8[0][kp][:, h, :], in_=e32)
                    nc.gpsimd.tensor_sub(out=e8[1][kp][:, h, :], in0=e32,
                                         in1=e8[0][kp][:, h, :])
                return e8

            def attend_stage(qb, e8):
                """A^T, rowsums, out for q-block qb."""
                a8 = [[None] * DP for _ in range(2)]
                for dc in range(D // 128):
                    pa = pca.tile([128, 512], F32, tag="pa", bufs=2)
                    n = 0
                    for (hx, he) in ((0, 0), (0, 1), (1, 0)):
                        for kp in range(KP):
                            nc.tensor.matmul(
                                pa,
                                xr8[hx][kp][:, :, dc * 128:(dc + 1) * 128],
                                e8[he][kp],
                                start=(n == 0), stop=(n == 3 * KP - 1),
                                perf_mode=DR)
                            n += 1
                    dp, h = dc // 2, dc % 2
                    if h == 0:
                        a8[0][dp] = a_pool.tile([128, 2, 512], E4, tag="a8",
                                                bufs=4 * DP,
                                                name=f"a8h_{qb}_{dp}")
                        a8[1][dp] = a_pool.tile([128, 2, 512], E4, tag="a8",
                                                bufs=4 * DP,
                                                name=f"a8l_{qb}_{dp}")
                    nc.scalar.activation(out=a8[0][dp][:, h, :], in_=pa,
                                         func=COPY, scale=C_A)
                    nc.vector.scalar_tensor_tensor(
                        out=a8[1][dp][:, h, :], in0=pa, scalar=C_A,
                        in1=a8[0][dp][:, h, :], op0=MULT, op1=SUBTRACT)
                # rowsums (over all ks) per q-chunk -> 1/(rowsum) * C_O
                rcs = []
                for qc in range(4):
                    pr = pcr.tile([128, 1], F32, tag="pr", bufs=1)
                    n = 0
                    for he in range(2):
                        for kp in range(KP):
                            nc.tensor.matmul(
                                pr,
                                e8[he][kp][:, :, qc * 128:(qc + 1) * 128],
                                ones8,
                                start=(n == 0), stop=(n == 2 * KP - 1),
                                perf_mode=DR)
                            n += 1
                    rc = small_pool.tile([128, 1], F32, tag="rc", bufs=16,
                                         name=f"rc_{qb}_{qc}")
                    nc.vector.reciprocal(out=rc, in_=pr)
                    rc2 = small_pool.tile([128, 1], F32, tag="rc2", bufs=16,
                                          name=f"rc2_{qb}_{qc}")
                    nc.scalar.activation(out=rc2, in_=rc, func=COPY, scale=C_O)
                    rcs.append(rc2)
                # out = A-contraction vs Wv, normalized
                for qc in range(4):
                    for oh in range(2):
                        po = pco.tile([128, 512], F32, tag="po", bufs=2)
                        n = 0
                        for (ha, hv) in ((0, 0), (0, 1), (1, 0)):
                            for dp in range(DP):
                                nc.tensor.matmul(
                                    po,
                                    a8[ha][dp][:, :, qc * 128:(qc + 1) * 128],
                                    wvt[hv][dp][:, :, oh * 512:(oh + 1) * 512],
                                    start=(n == 0), stop=(n == 3 * DP - 1),
                                    perf_mode=DR)
                                n += 1
                        os_ = outs_pool.tile([128, 512], F32, tag="outs",
                                             bufs=3)
                        nc.vector.tensor_scalar_mul(out=os_, in0=po,
                                                    scalar1=rcs[qc])
                        nc.sync.dma_start(
                            out=out_d[qb * 512 + qc * 128:
                                      qb * 512 + (qc + 1) * 128,
                                      oh * 512:(oh + 1) * 512],
                            in_=os_)

            # software pipeline: scores(qb) runs ahead of attend(qb-1)
            prev = None
            for qb in range(NSB):
                e8 = scores_stage(qb)
                if prev is not None:
                    attend_stage(qb - 1, prev)
                prev = e8
            attend_stage(NSB - 1, prev)

        top.close()

    nc.compile()
    return nc


def _split_np(a, scale):
    hi = (a * scale).astype(E4NP)
    lo = ((a * scale) - hi.astype(np.float32)).astype(E4NP)
    return hi, lo


def _prep_pairs(t):
    """[1024 d, 1024 c] fp8 -> [dp, p, h, c] with d = dp*256 + h*128 + p."""
    return np.ascontiguousarray(t.reshape(4, 2, 128, 1024).transpose(0, 2, 1, 3))


def prepare_weights(w):
    """Host-side weight prep: M = Wq Wk^T (f32) and hi/lo e4m3 pair tiles."""
    M = (w[0].astype(np.float64) @ w[1].T.astype(np.float64)).astype(np.float32)
    Mh, Ml = _split_np(M, SM)
    Vh, Vl = _split_np(w[2], SV)
    mprep = np.stack([_prep_pairs(Mh), _prep_pairs(Ml)])
    vprep = np.stack([_prep_pairs(Vh), _prep_pairs(Vl)])
    return mprep.view(np.uint8), vprep.view(np.uint8)


_NC_CACHE = {}


def _get_nc():
    if "full" not in _NC_CACHE:
        _NC_CACHE["full"] = build_attn()
    return _NC_CACHE["full"]


def kernel(**inputs):
    """Full-input entry point: x [8, 2048, 1024], kernel [3, 1024, 1024]."""
    from concourse.bass_utils import run_bass_kernel_spmd

    x = np.ascontiguousarray(inputs["x"], dtype=np.float32)
    w = np.ascontiguousarray(inputs["kernel"], dtype=np.float32)
    B = x.shape[0]
    mprep, vprep = prepare_weights(w)
    nc = _get_nc()
    in_maps = [{"x": x[b], "mprep": mprep, "vprep": vprep} for b in range(B)]
    res = run_bass_kernel_spmd(nc, in_maps, core_ids=list(range(B)))
    return np.stack([res.results[b]["out"] for b in range(B)], axis=0)


# revision 18
# speedup vs baseline: 9.9345x; 9.9345x over previous
"""Single-head attention Trainium2 kernel (batch=8 data-parallel over 8 cores).

Reference (per batch element): out = softmax((x Wq)(x Wk)^T / 8) (x Wv).

Strategy: fp8-e4m3 residual (hi+lo) arithmetic on the PE at DoubleRow rate
(0.5 cycles/row, 256-deep contraction per instruction), with all first-order
quantization error cancelled by 3-pass matmuls (hi*hi + lo*hi + hi*lo).
Measured end-to-end absmax_rel ~4e-3 (gate 2e-2).

PE weight loads are serial on this compile path (enable-ldw-opt=false), so
every matmul group is organized as one stationary -> four moving tiles
accumulating in four PSUM banks: the 256-row stationary load amortizes 4x.

Algebraic restructure: scores = x M x^T with M = Wq Wk^T precomputed on the
host (exact f32), so the device never materializes Q or K:
    G'^T = M^T-contraction vs x^T     [d', s]
    scoresT = x^T-contraction vs G'^T [ks, q]
    expT = exp(scores/8 - 5.5)        (global exp bias cancels in softmax)
    A^T  = x-contraction vs expT      [d, q]   (= (attn @ x)^T unnormalized)
    out  = A-contraction vs Wv, * 1/rowsum
Host also pre-splits M and Wv into hi/lo e4m3 pair-tile layouts (pure weight
preprocessing; x-dependent work all happens on device).

Power-of-2 scales keep every rescale exact: sx=16, sM=2048, sG=64, sWv=1024,
sA=1/4; hi and lo share one scale so all 3 residual passes accumulate in a
single PSUM group.
"""

import sys

sys.path.insert(0, "/opt/trn_rl_repo")

from contextlib import ExitStack

import numpy as np
import ml_dtypes

import concourse.bass as bass
import concourse.mybir as mybir
from concourse import bacc
from concourse.tile import TileContext
from concourse.masks import make_identity

F32 = mybir.dt.float32
F32R = mybir.dt.float32r
E4 = mybir.dt.float8e4
U8 = mybir.dt.uint8
DR = mybir.MatmulPerfMode.DoubleRow
EXP = mybir.ActivationFunctionType.Exp
COPY = mybir.ActivationFunctionType.Copy
MULT = mybir.AluOpType.mult
SUBTRACT = mybir.AluOpType.subtract

E4NP = ml_dtypes.float8_e4m3

S, D, O = 2048, 1024, 1024
SX = 16.0          # x scale
SM = 2048.0        # M scale
SG = 64.0          # G' scale
SV = 1024.0        # Wv scale
SA = 0.25          # A scale
BIAS = -5.5        # exp bias (cancels in softmax), keeps e4m3 in range
C_G = SG / (SX * SM)        # 2^-9  : psum(G'*sx*sM) -> G'*sG
C_E = (1.0 / 8.0) / (SX * SG)  # 2^-13: psum(scores*sx*sG) -> scores/8
C_A = SA / SX               # 2^-6  : psum(A*sx) -> A*sA
C_O = 1.0 / (SA * SV)       # 2^-8  : psum(out*sA*sV) -> out (pre-rowsum)

PASSES = ((0, 0), (0, 1), (1, 0))   # (hi,hi), (hi,lo), (lo,hi)


def build_attn(reps=1):
    """Bass module for one core: x[S,D] f32, mprep/vprep fp8 -> out[S,O] f32."""
    KC = S // 128      # 16 ks-chunks
    KP = KC // 2       # 8 ks-pairs
    DP = D // 256      # 4 d-pairs
    NSB = S // 512     # 4 s-blocks (also the q-blocks)

    nc = bacc.Bacc("TRN2", target_bir_lowering=False, debug=False)
    x_in = nc.dram_tensor("x", [S, D], F32, kind="ExternalInput")
    m_in = nc.dram_tensor("mprep", [2, DP, 128, 2, D], U8, kind="ExternalInput")
    v_in = nc.dram_tensor("vprep", [2, DP, 128, 2, O], U8, kind="ExternalInput")
    out_d = nc.dram_tensor("out", [S, O], F32, kind="ExternalOutput")

    with TileContext(nc) as tc:
      for _rep in range(reps):
        top = ExitStack()
        const_pool = top.enter_context(tc.tile_pool(name="constp", bufs=4))
        xr_pool = top.enter_context(tc.tile_pool(name="xrp", bufs=2 * KP))
        small_pool = top.enter_context(tc.tile_pool(name="smallp", bufs=16))

        ident_f = const_pool.tile([128, 128], F32, tag="identf")
        make_identity(nc, ident_f)
        ident_r = const_pool.tile([128, 128], F32R, tag="identr")
        nc.vector.tensor_copy(out=ident_r, in_=ident_f)
        ones8 = const_pool.tile([128, 2, 1], E4, tag="ones8")
        nc.gpsimd.memset(ones8, 1.0)
        bias_t = const_pool.tile([128, 1], F32, tag="biast")
        nc.gpsimd.memset(bias_t, BIAS)

        xr8 = [[None] * KP for _ in range(2)]   # [hl][kp] -> [128, 2, D]
        for hl in range(2):
            for kp in range(KP):
                xr8[hl][kp] = xr_pool.tile([128, 2, D], E4, tag="xr",
                                           bufs=2 * KP, name=f"xr_{hl}_{kp}")

        # e8 outlives the xt/gt pools -> reserved first (LIFO release)
        e_pool = top.enter_context(tc.tile_pool(name="ep", bufs=2 * KP * NSB))
        e8 = [[[None] * KP for _ in range(2)] for _ in range(NSB)]
        for qb in range(NSB):
            for hl in range(2):
                for kp in range(KP):
                    e8[qb][hl][kp] = e_pool.tile(
                        [128, 2, 512], E4, tag="e8", bufs=2 * KP * NSB,
                        name=f"e8_{qb}_{hl}_{kp}")

        phA = top.enter_context(ExitStack())
        phM = top.enter_context(ExitStack())
        if True:
            xt_pool = phA.enter_context(tc.tile_pool(name="xtp", bufs=2 * DP * NSB))
            gt_pool = phA.enter_context(tc.tile_pool(name="gtp", bufs=2 * DP * NSB))
            mst_pool = phM.enter_context(tc.tile_pool(name="mstp", bufs=2 * DP))
            mst = [[None] * DP for _ in range(2)]
            for hl in range(2):
                for dp in range(DP):
                    mst[hl][dp] = mst_pool.tile([128, 2, D], E4, tag="mst",
                                                bufs=2 * DP,
                                                name=f"mst_{hl}_{dp}")
            xt8 = [[[None] * NSB for _ in range(DP)] for _ in range(2)]
            gt8 = [[[None] * NSB for _ in range(DP)] for _ in range(2)]
            for hl in range(2):
                for dp in range(DP):
                    for sb in range(NSB):
                        xt8[hl][dp][sb] = xt_pool.tile(
                            [128, 2, 512], E4, tag="xt", bufs=2 * DP * NSB,
                            name=f"xt_{hl}_{dp}_{sb}")
                        gt8[hl][dp][sb] = gt_pool.tile(
                            [128, 2, 512], E4, tag="gt", bufs=2 * DP * NSB,
                            name=f"gt_{hl}_{dp}_{sb}")

            # ---- phase 1+2: x load, row-split, f32r transpose + xT split --
            with ExitStack() as ph1:
                xn_pool = ph1.enter_context(tc.tile_pool(name="xnp", bufs=6))
                pst = ph1.enter_context(
                    tc.tile_pool(name="pst", bufs=3, space="PSUM"))
                for sb in range(NSB):
                    xns = []
                    for ss in range(4):
                        kc = sb * 4 + ss
                        xn = xn_pool.tile([128, D], F32R, tag="xn", bufs=6)
                        dma_eng = nc.sync if kc % 2 == 0 else nc.gpsimd
                        for colh in range(2):
                            dma_eng.dma_start(
                                out=xn[:, colh * 512:(colh + 1) * 512],
                                in_=x_in[kc * 128:(kc + 1) * 128,
                                         colh * 512:(colh + 1) * 512
                                         ].bitcast(F32R))
                        xns.append(xn)
                        kp, h = kc // 2, kc % 2
                        nc.scalar.activation(out=xr8[0][kp][:, h, :],
                                             in_=xn.bitcast(F32), func=COPY,
                                             scale=SX)
                        nc.vector.scalar_tensor_tensor(
                            out=xr8[1][kp][:, h, :], in0=xn.bitcast(F32),
                            scalar=SX, in1=xr8[0][kp][:, h, :],
                            op0=MULT, op1=SUBTRACT)
                    if sb == 1:
                        for hl in range(2):
                            for dp in range(DP):
                                eng = nc.sync if dp % 2 == 0 else nc.gpsimd
                                eng.dma_start(out=mst[hl][dp],
                                              in_=m_in[hl, dp].bitcast(E4))
                    for dc in range(D // 128):
                        pt = pst.tile([128, 512], F32R, tag="pt", bufs=3)
                        for ss in range(4):
                            nc.tensor.transpose(
                                pt[:, ss * 128:(ss + 1) * 128],
                                xns[ss][:, dc * 128:(dc + 1) * 128],
                                ident_r)
                        ptf = pt.bitcast(F32)
                        dh = xt8[0][dc // 2][sb]
                        dl = xt8[1][dc // 2][sb]
                        nc.scalar.activation(out=dh[:, dc % 2, :], in_=ptf,
                                             func=COPY, scale=SX)
                        nc.vector.scalar_tensor_tensor(
                            out=dl[:, dc % 2, :], in0=ptf, scalar=SX,
                            in1=dh[:, dc % 2, :], op0=MULT, op1=SUBTRACT)

            # ---- phase 3: G'^T, one stationary -> 4 s-blocks ----
            with ExitStack() as ph3:
                psg = ph3.enter_context(
                    tc.tile_pool(name="psg", bufs=2 * NSB, space="PSUM"))
                for dpc in range(D // 128):
                    pgs = [psg.tile([128, 512], F32, tag="pg", bufs=2 * NSB,
                                    name=f"pg_{dpc}_{sb}") for sb in range(NSB)]
                    n = 0
                    for (hm, hx) in PASSES:
                        for dp in range(DP):
                            st = mst[hm][dp][:, :, dpc * 128:(dpc + 1) * 128]
                            for sb in range(NSB):
                                nc.tensor.matmul(
                                    pgs[sb], st, xt8[hx][dp][sb],
                                    start=(n == 0), stop=(n == 3 * DP - 1),
                                    perf_mode=DR)
                            n += 1
                    for sb in range(NSB):
                        dh = gt8[0][dpc // 2][sb]
                        dl = gt8[1][dpc // 2][sb]
                        nc.scalar.activation(out=dh[:, dpc % 2, :],
                                             in_=pgs[sb], func=COPY, scale=C_G)
                        nc.vector.scalar_tensor_tensor(
                            out=dl[:, dpc % 2, :], in0=pgs[sb], scalar=C_G,
                            in1=dh[:, dpc % 2, :], op0=MULT, op1=SUBTRACT)

            phM.close()   # frees M tiles after phase 3
            # ---- phase 4a: scoresT + exp for all q-blocks, kc-major ----
            with ExitStack() as ph4a:
                e32_pool = ph4a.enter_context(tc.tile_pool(name="e32p", bufs=4))
                pcs = ph4a.enter_context(
                    tc.tile_pool(name="pcs", bufs=2 * NSB, space="PSUM"))
                for kc in range(KC):
                    sbk, ss = kc // 4, kc % 4
                    pss = [pcs.tile([128, 512], F32, tag="ps", bufs=2 * NSB,
                                    name=f"ps_{kc}_{qb}") for qb in range(NSB)]
                    n = 0
                    for (ha, hb) in PASSES:
                        for dp in range(DP):
                            st = xt8[ha][dp][sbk][:, :, ss * 128:(ss + 1) * 128]
                            for qb in range(NSB):
                                nc.tensor.matmul(
                                    pss[qb], st, gt8[hb][dp][qb],
                                    start=(n == 0), stop=(n == 3 * DP - 1),
                                    perf_mode=DR)
                            n += 1
                    kp, h = kc // 2, kc % 2
                    for qb in range(NSB):
                        e32 = e32_pool.tile([128, 512], F32, tag="e32", bufs=4)
                        nc.scalar.activation(out=e32, in_=pss[qb], func=EXP,
                                             bias=bias_t, scale=C_E)
                        nc.gpsimd.tensor_copy(out=e8[qb][0][kp][:, h, :],
                                              in_=e32)
                        nc.gpsimd.tensor_sub(out=e8[qb][1][kp][:, h, :],
                                             in0=e32,
                                             in1=e8[qb][0][kp][:, h, :])
        phA.close()   # frees mst/xt/gt SBUF before a8 is reserved

        # ---- phase 4b: rowsums (expT-slice stationary, ones moving) ----
        rcs = [[None] * 4 for _ in range(NSB)]   # [qb][qc] -> rc2 [128,1]
        with ExitStack() as ph4b:
            pcr = ph4b.enter_context(
                tc.tile_pool(name="pcr", bufs=2, space="PSUM"))
            for qb in range(NSB):
                for qc in range(4):
                    pr = pcr.tile([128, 1], F32, tag="pr", bufs=2)
                    n = 0
                    for he in range(2):
                        for kp in range(KP):
                            nc.tensor.matmul(
                                pr,
                                e8[qb][he][kp][:, :, qc * 128:(qc + 1) * 128],
                                ones8,
                                start=(n == 0), stop=(n == 2 * KP - 1),
                                perf_mode=DR)
                            n += 1
                    rc = small_pool.tile([128, 1], F32, tag="rc", bufs=16,
                                         name=f"rc_{qb}_{qc}")
                    nc.vector.reciprocal(out=rc, in_=pr)
                    rc2 = small_pool.tile([128, 1], F32, tag="rc2", bufs=16,
                                          name=f"rc2_{qb}_{qc}")
                    nc.scalar.activation(out=rc2, in_=rc, func=COPY, scale=C_O)
                    rcs[qb][qc] = rc2

        # ---- phase 4c: A^T for all q-blocks, dc-major ----
        a_pool = top.enter_context(tc.tile_pool(name="ap", bufs=2 * DP * NSB))
        a8 = [[[None] * DP for _ in range(2)] for _ in range(NSB)]
        for qb in range(NSB):
            for hl in range(2):
                for dp in range(DP):
                    a8[qb][hl][dp] = a_pool.tile(
                        [128, 2, 512], E4, tag="a8", bufs=2 * DP * NSB,
                        name=f"a8_{qb}_{hl}_{dp}")
        wv_pool = top.enter_context(tc.tile_pool(name="wvp", bufs=2 * DP))
        wvt = [[None] * DP for _ in range(2)]
        for hl in range(2):
            for dp in range(DP):
                wvt[hl][dp] = wv_pool.tile([128, 2, O], E4, tag="wvt",
                                           bufs=2 * DP, name=f"wvt_{hl}_{dp}")
                eng = nc.sync if dp % 2 == 0 else nc.gpsimd
                eng.dma_start(out=wvt[hl][dp], in_=v_in[hl, dp].bitcast(E4))
        with ExitStack() as ph4c:
            pca = ph4c.enter_context(
                tc.tile_pool(name="pca", bufs=2 * NSB, space="PSUM"))
            for dc in range(D // 128):
                pas = [pca.tile([128, 512], F32, tag="pa", bufs=2 * NSB,
                                name=f"pa_{dc}_{qb}") for qb in range(NSB)]
                n = 0
                for (hx, he) in PASSES:
                    for kp in range(KP):
                        st = xr8[hx][kp][:, :, dc * 128:(dc + 1) * 128]
                        for qb in range(NSB):
                            nc.tensor.matmul(
                                pas[qb], st, e8[qb][he][kp],
                                start=(n == 0), stop=(n == 3 * KP - 1),
                                perf_mode=DR)
                        n += 1
                dp, h = dc // 2, dc % 2
                for qb in range(NSB):
                    ah = a8[qb][0][dp]
                    al = a8[qb][1][dp]
                    nc.scalar.activation(out=ah[:, h, :], in_=pas[qb],
                                         func=COPY, scale=C_A)
                    nc.vector.scalar_tensor_tensor(
                        out=al[:, h, :], in0=pas[qb], scalar=C_A,
                        in1=ah[:, h, :], op0=MULT, op1=SUBTRACT)

        # ---- phase 4d: out = A-contraction vs Wv, normalized ----
        with ExitStack() as ph4d:
            outs_pool = ph4d.enter_context(tc.tile_pool(name="outsp", bufs=4))
            pco = ph4d.enter_context(
                tc.tile_pool(name="pco", bufs=4, space="PSUM"))
            for qb in range(NSB):
                for qc in range(4):
                    pos = [pco.tile([128, 512], F32, tag="po", bufs=4,
                                    name=f"po_{qb}_{qc}_{oh}")
                           for oh in range(2)]
                    n = 0
                    for (ha, hv) in PASSES:
                        for dp in range(DP):
                            st = a8[qb][ha][dp][:, :, qc * 128:(qc + 1) * 128]
                            for oh in range(2):
                                nc.tensor.matmul(
                                    pos[oh], st,
                                    wvt[hv][dp][:, :, oh * 512:(oh + 1) * 512],
                                    start=(n == 0), stop=(n == 3 * DP - 1),
                                    perf_mode=DR)
                            n += 1
                    for oh in range(2):
                        os_ = outs_pool.tile([128, 512], F32, tag="outs",
                                             bufs=4)
                        nc.vector.tensor_scalar_mul(out=os_, in0=pos[oh],
                                                    scalar1=rcs[qb][qc])
                        nc.sync.dma_start(
                            out=out_d[qb * 512 + qc * 128:
                                      qb * 512 + (qc + 1) * 128,
                                      oh * 512:(oh + 1) * 512],
                            in_=os_)

        top.close()

    nc.compile()
    return nc


def _split_np(a, scale):
    hi = (a * scale).astype(E4NP)
    lo = ((a * scale) - hi.astype(np.float32)).astype(E4NP)
    return hi, lo


def _prep_pairs(t):
    """[1024 d, 1024 c] fp8 -> [dp, p, h, c] with d = dp*256 + h*128 + p."""
    return np.ascontiguousarray(t.reshape(4, 2, 128, 1024).transpose(0, 2, 1, 3))


def prepare_weights(w):
    """Host-side weight prep: M = Wq Wk^T (f32) and hi/lo e4m3 pair tiles."""
    M = (w[0].astype(np.float64) @ w[1].T.astype(np.float64)).astype(np.float32)
    Mh, Ml = _split_np(M, SM)
    Vh, Vl = _split_np(w[2], SV)
    mprep = np.stack([_prep_pairs(Mh), _prep_pairs(Ml)])
    vprep = np.stack([_prep_pairs(Vh), _prep_pairs(Vl)])
    return mprep.view(np.uint8), vprep.view(np.uint8)


_NC_CACHE = {}


def _get_nc():
    if "full" not in _NC_CACHE:
        _NC_CACHE["full"] = build_attn()
    return _NC_CACHE["full"]


def kernel(**inputs):
    """Full-input entry point: x [8, 2048, 1024], kernel [3, 1024, 1024]."""
    from concourse.bass_utils import run_bass_kernel_spmd

    x = np.ascontiguousarray(inputs["x"], dtype=np.float32)
    w = np.ascontiguousarray(inputs["kernel"], dtype=np.float32)
    B = x.shape[0]
    mprep, vprep = prepare_weights(w)
    nc = _get_nc()
    in_maps = [{"x": x[b], "mprep": mprep, "vprep": vprep} for b in range(B)]
    res = run_bass_kernel_spmd(nc, in_maps, core_ids=list(range(B)))
    return np.stack([res.results[b]["out"] for b in range(B)], axis=0)


# revision 19
# speedup vs baseline: 10.9605x; 1.1033x over previous
"""Single-head attention Trainium2 kernel (batch=8 data-parallel over 8 cores).

Reference computation (per batch element b):
    Q = x @ Wq; K = x @ Wk; V = x @ Wv          (x: [S, D], W*: [D, O])
    out = softmax(Q @ K.T * SCALE) @ V          (SCALE = 1/8, hardcoded sqrt(64))

Kernel strategy (per core, one batch element), all matmuls f32r:
  Host precomputes M = Wq @ Wk^T (exact f32), so scores = (x M) x^T and the
  device never materializes Q or K separately:
  Phase A: x -> xT via PE transposes (d on partitions).
  Phase B: G'^T = (xM)^T streamed to DRAM scratch (plays the role of Q^T);
           "K^T" is just xT, copied to phase-C-lived tiles. This removes the
           entire K projection (one of the three S*D*O matmuls) versus the
           plain QKV formulation.
  Phase C (per q-block): scoresT[ks,q] = xT-chunks @ G'T-block,
           expT = exp(SCALE*scoresT) (ACT, fused scale),
           row-sums via ones-matmul -> reciprocal,
           A^T[d,q] = x-chunks.T @ expT (reassociation: attn @ x),
           out[q,o] = (A^T).T @ Wv, normalized by reciprocal on eviction.
"""
import sys
sys.path.insert(0, "/opt/trn_rl_repo")
from contextlib import ExitStack
import numpy as np
import concourse.bass as bass
import concourse.mybir as mybir
from concourse import bacc
from concourse.tile import TileContext
from concourse.masks import make_identity

F32 = mybir.dt.float32
F32R = mybir.dt.float32r
EXP = mybir.ActivationFunctionType.Exp
SCALE = 1.0 / 8.0


def build_attn(S=2048, D=1024, O=1024, QB=256, compute_dtype=F32R, reps=1, phases='abcuvo'):
    CD = compute_dtype
    SB = 512
    NSB = S // SB
    DC = D // 128
    OC = O // 128
    KC = S // 128
    NQB = S // QB
    QC = QB // 128
    OH = (O + 511) // 512
    OHW = min(O, 512)

    nc = bacc.Bacc("TRN2", target_bir_lowering=False, debug=False)
    x_in = nc.dram_tensor("x", [S, D], F32, kind="ExternalInput")
    m_in = nc.dram_tensor("m", [D, O], F32, kind="ExternalInput")
    wv_in = nc.dram_tensor("wv", [D, O], F32, kind="ExternalInput")
    out_d = nc.dram_tensor("out", [S, O], F32, kind="ExternalOutput")

    def cast(ap):
        return ap.bitcast(CD) if CD != F32 else ap

    with TileContext(nc) as tc:
      for _rep in range(reps):
        top = ExitStack()
        dram = top.enter_context(tc.tile_pool(name="dram", bufs=1, space="DRAM"))
        qt_t = dram.tile([O, S], F32)

        kt_pool = top.enter_context(tc.tile_pool(name="ktp", bufs=OC * NSB))
        const_pool = top.enter_context(tc.tile_pool(name="constp", bufs=1))

        ident_f = const_pool.tile([128, 128], F32, tag="identf")
        make_identity(nc, ident_f)
        if CD != F32:
            ident = const_pool.tile([128, 128], CD, tag="identr")
            nc.vector.tensor_copy(out=ident, in_=ident_f)
        else:
            ident = ident_f
        ones_f = const_pool.tile([128, 1], F32, tag="onesf")
        nc.gpsimd.memset(ones_f, 1.0)
        if CD != F32:
            ones = const_pool.tile([128, 1], CD, tag="onesr")
            nc.vector.tensor_copy(out=ones, in_=ones_f)
        else:
            ones = ones_f

        kt = [[None] * NSB for _ in range(OC)]

        with ExitStack() as ph_ab:
            xn_pool = ph_ab.enter_context(tc.tile_pool(name="xnp", bufs=6))
            xt_pool = ph_ab.enter_context(tc.tile_pool(name="xtp", bufs=DC * NSB))
            w_pool = ph_ab.enter_context(tc.tile_pool(name="wp", bufs=4 * DC))
            qs_pool = ph_ab.enter_context(tc.tile_pool(name="qsp", bufs=4))
            psA = ph_ab.enter_context(tc.tile_pool(name="psA", bufs=4, space="PSUM"))
            psB = ph_ab.enter_context(tc.tile_pool(name="psB", bufs=4, space="PSUM"))

            xt = [[None] * NSB for _ in range(DC)]
            xn_sb = [None] * (SB // 128)
            _mid = (NSB + 1) // 2
            _halves = [h for h in (list(range(0, _mid)), list(range(_mid, NSB))) if h]
            for half in range(len(_halves) if 'b' in phases else 0):
                sbs = _halves[half]
                for sb in sbs:
                    for ss in range(SB // 128):
                        kc = sb * (SB // 128) + ss
                        xn_t = xn_pool.tile([128, D], CD, tag="xn", bufs=6)
                        dma_eng = nc.sync if kc % 2 == 0 else nc.gpsimd
                        dma_eng.dma_start(
                            out=xn_t, in_=cast(x_in[kc * 128 : (kc + 1) * 128, :])
                        )
                        xn_sb[ss] = xn_t
                    for dc in range(DC):
                        ps = psA.tile([128, SB], CD, tag="pst", bufs=4)
                        for s2 in range(SB // 128):
                            nc.tensor.transpose(
                                ps[:, s2 * 128 : (s2 + 1) * 128],
                                xn_sb[s2][:, dc * 128 : (dc + 1) * 128],
                                ident,
                            )
                        xt[dc][sb] = xt_pool.tile(
                            [128, SB], CD, tag="xt", bufs=DC * NSB, name=f"xt_{dc}_{sb}"
                        )
                        nc.vector.tensor_copy(out=xt[dc][sb], in_=ps)
                for oc in range(OC):
                    wq_t = w_pool.tile(
                        [128, DC * 128], CD, tag="wq", bufs=3, name=f"wq_{half}_{oc}"
                    )
                    nc.sync.dma_start(
                        out=wq_t.rearrange("p (c o) -> p c o", c=DC),
                        in_=cast(m_in[:, oc * 128 : (oc + 1) * 128]).rearrange(
                            "(c p) o -> p c o", p=128
                        ),
                    )
                    wq_col = [wq_t[:, dc * 128 : (dc + 1) * 128] for dc in range(DC)]
                    qs = qs_pool.tile(
                        [128, SB * len(sbs)], F32, tag="qts", bufs=2, name=f"qts_{half}_{oc}"
                    )
                    for j, sb in enumerate(sbs):
                        ps_q = psB.tile([128, SB], F32, tag="psb", bufs=4)
                        for dc in range(DC):
                            nc.tensor.matmul(
                                ps_q, wq_col[dc], xt[dc][sb],
                                start=(dc == 0), stop=(dc == DC - 1),
                            )
                        nc.vector.tensor_copy(
                            out=qs[:, j * SB : (j + 1) * SB], in_=ps_q
                        )
                        # "K^T" for scores is just x^T: keep fp-rounded copy
                        if kt[oc][sb] is None:
                            kt_tile = kt_pool.tile(
                                [128, SB], CD, tag="kt", bufs=OC * NSB,
                                name=f"kt_{oc}_{sb}"
                            )
                            nc.scalar.copy(out=kt_tile, in_=xt[oc][sb])
                            kt[oc][sb] = kt_tile
                    nc.sync.dma_start(
                        out=qt_t[
                            oc * 128 : (oc + 1) * 128,
                            sbs[0] * SB : (sbs[-1] + 1) * SB,
                        ],
                        in_=qs,
                    )

        with ExitStack() as ph_c:
          if 'c' in phases:
                xn2_pool = ph_c.enter_context(tc.tile_pool(name="xn2p", bufs=KC))
                wv_pool = ph_c.enter_context(tc.tile_pool(name="wvp", bufs=DC))
                qtin_pool = ph_c.enter_context(tc.tile_pool(name="qtinp", bufs=OC + 2))
                exp_pool = ph_c.enter_context(tc.tile_pool(name="expp", bufs=KC + 1))
                at_pool = ph_c.enter_context(tc.tile_pool(name="atp", bufs=DC))
                outs_pool = ph_c.enter_context(tc.tile_pool(name="outsp", bufs=2))
                small_pool = ph_c.enter_context(tc.tile_pool(name="smallp", bufs=4 * QC))
                pcs = ph_c.enter_context(tc.tile_pool(name="pcs", bufs=3, space="PSUM"))
                pcsum = ph_c.enter_context(tc.tile_pool(name="pcsum", bufs=1, space="PSUM"))
                pca = ph_c.enter_context(tc.tile_pool(name="pca", bufs=2, space="PSUM"))
                pco = ph_c.enter_context(tc.tile_pool(name="pco", bufs=2, space="PSUM"))

                xn2 = []
                for kc in range(KC):
                    t = xn2_pool.tile([128, D], CD, tag="xn2", bufs=KC, name=f"xn2_{kc}")
                    nc.gpsimd.dma_start(out=t, in_=cast(x_in[kc * 128 : (kc + 1) * 128, :]))
                    xn2.append(t)
                wv = []
                for dc in range(DC):
                    t = wv_pool.tile([128, O], CD, tag="wv", bufs=DC, name=f"wv_{dc}")
                    nc.gpsimd.dma_start(out=t, in_=cast(wv_in[dc * 128 : (dc + 1) * 128, :]))
                    wv.append(t)

                for qb in range(NQB):
                    q0 = qb * QB
                    qt_blk = qtin_pool.tile(
                        [128, OC * QB], CD, tag="qtin", bufs=2, name=f"qtin_{qb}"
                    )
                    if qb == 0:
                        for oc in range(OC):
                            nc.sync.dma_start(
                                out=qt_blk[:, oc * QB : (oc + 1) * QB],
                                in_=cast(
                                    qt_t[oc * 128 : (oc + 1) * 128, q0 : q0 + QB]
                                ),
                            )
                    else:
                        nc.sync.dma_start(
                            out=qt_blk.rearrange("p (c q) -> p c q", c=OC),
                            in_=cast(qt_t[:, q0 : q0 + QB]).rearrange(
                                "(c p) q -> p c q", p=128
                            ),
                        )
                    qts = [qt_blk[:, oc * QB : (oc + 1) * QB] for oc in range(OC)]
                    exp_pairs = []
                    for kp in range(KC // 2):
                        ps_s = pcs.tile([128, 2 * QB], F32, tag="pcs", bufs=3)
                        for half in range(2):
                            kc = 2 * kp + half
                            sb, ss = kc // (SB // 128), kc % (SB // 128)
                            dst = ps_s[:, half * QB : (half + 1) * QB]
                            for oc in range(OC):
                                nc.tensor.matmul(
                                    dst,
                                    kt[oc][sb][:, ss * 128 : (ss + 1) * 128],
                                    qts[oc],
                                    start=(oc == 0), stop=(oc == OC - 1),
                                )
                        e = exp_pool.tile([128, 2 * QB], CD, tag="expT", bufs=KC // 2 + 1)
                        nc.scalar.activation(out=e, in_=ps_s, func=EXP, scale=SCALE)
                        exp_pairs.append(e)
                    expT = [
                        exp_pairs[kc // 2][:, (kc % 2) * QB : (kc % 2 + 1) * QB]
                        for kc in range(KC)
                    ]
                    aT_pairs = []
                    for dp in range((DC if 'v' in phases else 0) // 2):
                        ps_a = pca.tile([128, 2 * QB], F32, tag="pca", bufs=2)
                        for half in range(2):
                            dc = 2 * dp + half
                            dst = ps_a[:, half * QB : (half + 1) * QB]
                            for kc in range(KC):
                                nc.tensor.matmul(
                                    dst,
                                    xn2[kc][:, dc * 128 : (dc + 1) * 128],
                                    expT[kc],
                                    start=(kc == 0), stop=(kc == KC - 1),
                                )
                        a_t = at_pool.tile([128, 2 * QB], CD, tag="aT", bufs=DC // 2)
                        nc.vector.tensor_copy(out=a_t, in_=ps_a)
                        aT_pairs.append(a_t)
                    aT = [
                        aT_pairs[dc // 2][:, (dc % 2) * QB : (dc % 2 + 1) * QB]
                        for dc in range(DC if 'v' in phases else 0)
                    ]
                    recips = []
                    for qc in range(QC if 'u' in phases else 0):
                        ps_sum = pcsum.tile([128, 1], F32, tag="pcsum", bufs=1)
                        for kc in range(KC):
                            nc.tensor.matmul(
                                ps_sum,
                                expT[kc][:, qc * 128 : (qc + 1) * 128].bitcast(F32),
                                ones_f,
                                start=(kc == 0), stop=(kc == KC - 1),
                            )
                        rc = small_pool.tile([128, 1], F32, tag="recip", bufs=4 * QC)
                        nc.vector.reciprocal(out=rc, in_=ps_sum)
                        recips.append(rc)
                    for qc in range(QC if 'o' in phases else 0):
                        for oh in range(OH):
                            ps_o = pco.tile([128, OHW], F32, tag="pco", bufs=2)
                            for dc in range(DC):
                                nc.tensor.matmul(
                                    ps_o,
                                    aT[dc][:, qc * 128 : (qc + 1) * 128],
                                    wv[dc][:, oh * OHW : (oh + 1) * OHW],
                                    start=(dc == 0), stop=(dc == DC - 1),
                                )
                            os_ = outs_pool.tile([128, OHW], F32, tag="outs", bufs=2)
                            nc.vector.tensor_scalar_mul(out=os_, in0=ps_o, scalar1=recips[qc])
                            nc.sync.dma_start(
                                out=out_d[
                                    q0 + qc * 128 : q0 + (qc + 1) * 128,
                                    oh * OHW : (oh + 1) * 512,
                                ],
                                in_=os_,
                            )

        top.close()

    nc.compile()
    return nc


_NC_CACHE = {}


def _get_nc():
    key = "full"
    if key not in _NC_CACHE:
        _NC_CACHE[key] = build_attn()
    return _NC_CACHE[key]


def prepare_weights(w):
    """Host-side weight prep: M = Wq Wk^T (f32) and Wv, both [D, O]."""
    m = (w[0].astype(np.float64) @ w[1].T.astype(np.float64)).astype(np.float32)
    return np.ascontiguousarray(m), np.ascontiguousarray(w[2], np.float32)


def kernel(**inputs):
    """Full-input entry point: x [8, 2048, 1024], kernel [3, 1024, 1024]."""
    from concourse.bass_utils import run_bass_kernel_spmd

    x = np.ascontiguousarray(inputs["x"], dtype=np.float32)
    w = np.ascontiguousarray(inputs["kernel"], dtype=np.float32)
    B = x.shape[0]
    m, wv = prepare_weights(w)
    nc = _get_nc()
    in_maps = [{"x": x[b], "m": m, "wv": wv} for b in range(B)]
    res = run_bass_kernel_spmd(nc, in_maps, core_ids=list(range(B)))
    return np.stack([res.results[b]["out"] for b in range(B)], axis=0)


# revision 20
# speedup vs baseline: 13.2409x; 1.2081x over previous
"""Single-head attention Trainium2 kernel (batch=8 data-parallel over 8 cores).

Reference computation (per batch element b):
    Q = x @ Wq; K = x @ Wk; V = x @ Wv          (x: [S, D], W*: [D, O])
    out = softmax(Q @ K.T * SCALE) @ V          (SCALE = 1/8, hardcoded sqrt(64))

Kernel strategy (per core, one batch element), all matmuls f32r:
  Host precomputes M = Wq @ Wk^T (exact f32), so scores = (x M) x^T and the
  device never materializes Q or K separately:
  Phase A: x -> xT via PE transposes (d on partitions).
  Phase B: G'^T = (xM)^T streamed to DRAM scratch (plays the role of Q^T);
           "K^T" is just xT, copied to phase-C-lived tiles. This removes the
           entire K projection (one of the three S*D*O matmuls) versus the
           plain QKV formulation.
  Phase C (per q-block): scoresT[ks,q] = xT-chunks @ G'T-block,
           expT = exp(SCALE*scoresT) (ACT, fused scale),
           row-sums via ones-matmul -> reciprocal,
           A^T[d,q] = x-chunks.T @ expT (reassociation: attn @ x),
           out[q,o] = (A^T).T @ Wv, normalized by reciprocal on eviction.
"""
import sys
sys.path.insert(0, "/opt/trn_rl_repo")
from contextlib import ExitStack
import numpy as np
import concourse.bass as bass
import concourse.mybir as mybir
from concourse import bacc
from concourse.tile import TileContext
from concourse.masks import make_identity

F32 = mybir.dt.float32
F32R = mybir.dt.float32r
BF16 = mybir.dt.bfloat16
EXP = mybir.ActivationFunctionType.Exp
SCALE = 1.0 / 8.0


def build_attn(S=2048, D=1024, O=1024, QB=512, compute_dtype=F32R, reps=1, phases='abcuvo'):
    CD = compute_dtype
    SB = 512
    NSB = S // SB
    DC = D // 128
    OC = O // 128
    KC = S // 128
    NQB = S // QB
    QC = QB // 128
    OH = (O + 511) // 512
    OHW = min(O, 512)

    nc = bacc.Bacc("TRN2", target_bir_lowering=False, debug=False)
    x_in = nc.dram_tensor("x", [S, D], F32, kind="ExternalInput")
    m_in = nc.dram_tensor("m", [D, O], F32, kind="ExternalInput")
    wv_in = nc.dram_tensor("wv", [D, O], F32, kind="ExternalInput")
    out_d = nc.dram_tensor("out", [S, O], F32, kind="ExternalOutput")

    def cast(ap):
        return ap.bitcast(CD) if CD != F32 else ap

    with TileContext(nc) as tc:
      for _rep in range(reps):
        top = ExitStack()
        dram = top.enter_context(tc.tile_pool(name="dram", bufs=1, space="DRAM"))
        qt_t = dram.tile([O, S], F32)

        kt_pool = top.enter_context(tc.tile_pool(name="ktp", bufs=OC * NSB))
        const_pool = top.enter_context(tc.tile_pool(name="constp", bufs=1))

        ident_f = const_pool.tile([128, 128], F32, tag="identf")
        make_identity(nc, ident_f)
        if CD != F32:
            ident = const_pool.tile([128, 128], CD, tag="identr")
            nc.vector.tensor_copy(out=ident, in_=ident_f)
        else:
            ident = ident_f
        ones_f = const_pool.tile([128, 1], F32, tag="onesf")
        nc.gpsimd.memset(ones_f, 1.0)
        ones_b = const_pool.tile([128, 1], BF16, tag="onesb")
        nc.gpsimd.memset(ones_b, 1.0)

        kt = [[None] * NSB for _ in range(OC)]

        with ExitStack() as ph_ab:
            xn_pool = ph_ab.enter_context(tc.tile_pool(name="xnp", bufs=6))
            xt_pool = ph_ab.enter_context(tc.tile_pool(name="xtp", bufs=DC * NSB))
            w_pool = ph_ab.enter_context(tc.tile_pool(name="wp", bufs=4 * DC))
            qs_pool = ph_ab.enter_context(tc.tile_pool(name="qsp", bufs=4))
            psA = ph_ab.enter_context(tc.tile_pool(name="psA", bufs=4, space="PSUM"))
            psB = ph_ab.enter_context(tc.tile_pool(name="psB", bufs=4, space="PSUM"))

            xt = [[None] * NSB for _ in range(DC)]
            xn_sb = [None] * (SB // 128)
            _mid = (NSB + 1) // 2
            _halves = [h for h in (list(range(0, _mid)), list(range(_mid, NSB))) if h]
            for half in range(len(_halves) if 'b' in phases else 0):
                sbs = _halves[half]
                for sb in sbs:
                    for ss in range(SB // 128):
                        kc = sb * (SB // 128) + ss
                        xn_t = xn_pool.tile([128, D], CD, tag="xn", bufs=6)
                        dma_eng = nc.sync if kc % 2 == 0 else nc.gpsimd
                        dma_eng.dma_start(
                            out=xn_t, in_=cast(x_in[kc * 128 : (kc + 1) * 128, :])
                        )
                        xn_sb[ss] = xn_t
                    for dc in range(DC):
                        ps = psA.tile([128, SB], CD, tag="pst", bufs=4)
                        for s2 in range(SB // 128):
                            nc.tensor.transpose(
                                ps[:, s2 * 128 : (s2 + 1) * 128],
                                xn_sb[s2][:, dc * 128 : (dc + 1) * 128],
                                ident,
                            )
                        xt[dc][sb] = xt_pool.tile(
                            [128, SB], CD, tag="xt", bufs=DC * NSB, name=f"xt_{dc}_{sb}"
                        )
                        nc.vector.tensor_copy(out=xt[dc][sb], in_=ps)
                for oc in range(OC):
                    wq_t = w_pool.tile(
                        [128, DC * 128], CD, tag="wq", bufs=3, name=f"wq_{half}_{oc}"
                    )
                    nc.sync.dma_start(
                        out=wq_t.rearrange("p (c o) -> p c o", c=DC),
                        in_=cast(m_in[:, oc * 128 : (oc + 1) * 128]).rearrange(
                            "(c p) o -> p c o", p=128
                        ),
                    )
                    wq_col = [wq_t[:, dc * 128 : (dc + 1) * 128] for dc in range(DC)]
                    qs = qs_pool.tile(
                        [128, SB * len(sbs)], F32, tag="qts", bufs=2, name=f"qts_{half}_{oc}"
                    )
                    for j, sb in enumerate(sbs):
                        ps_q = psB.tile([128, SB], F32, tag="psb", bufs=4)
                        for dc in range(DC):
                            nc.tensor.matmul(
                                ps_q, wq_col[dc], xt[dc][sb],
                                start=(dc == 0), stop=(dc == DC - 1),
                            )
                        nc.vector.tensor_copy(
                            out=qs[:, j * SB : (j + 1) * SB], in_=ps_q
                        )
                        # "K^T" for scores is just x^T: keep fp-rounded copy
                        if kt[oc][sb] is None:
                            kt_tile = kt_pool.tile(
                                [128, SB], CD, tag="kt", bufs=OC * NSB,
                                name=f"kt_{oc}_{sb}"
                            )
                            nc.scalar.copy(out=kt_tile, in_=xt[oc][sb])
                            kt[oc][sb] = kt_tile
                    nc.sync.dma_start(
                        out=qt_t[
                            oc * 128 : (oc + 1) * 128,
                            sbs[0] * SB : (sbs[-1] + 1) * SB,
                        ],
                        in_=qs,
                    )

        with ExitStack() as ph_c:
          if 'c' in phases:
                xn2_pool = ph_c.enter_context(tc.tile_pool(name="xn2p", bufs=KC))
                wv_pool = ph_c.enter_context(tc.tile_pool(name="wvp", bufs=DC))
                qtin_pool = ph_c.enter_context(tc.tile_pool(name="qtinp", bufs=OC + 2))
                exp_pool = ph_c.enter_context(tc.tile_pool(name="expp", bufs=KC + 1))
                at_pool = ph_c.enter_context(tc.tile_pool(name="atp", bufs=DC))
                outs_pool = ph_c.enter_context(tc.tile_pool(name="outsp", bufs=2))
                small_pool = ph_c.enter_context(tc.tile_pool(name="smallp", bufs=4 * QC))
                pcs = ph_c.enter_context(tc.tile_pool(name="pcs", bufs=3, space="PSUM"))
                pcsum = ph_c.enter_context(tc.tile_pool(name="pcsum", bufs=1, space="PSUM"))
                pca = ph_c.enter_context(tc.tile_pool(name="pca", bufs=2, space="PSUM"))
                pco = ph_c.enter_context(tc.tile_pool(name="pco", bufs=2, space="PSUM"))

                xn2 = []
                for kc in range(KC):
                    t = xn2_pool.tile([128, D], BF16, tag="xn2", bufs=KC, name=f"xn2_{kc}")
                    nc.gpsimd.dma_start(out=t, in_=x_in[kc * 128 : (kc + 1) * 128, :])
                    xn2.append(t)
                wv = []
                for dc in range(DC):
                    t = wv_pool.tile([128, O], CD, tag="wv", bufs=DC, name=f"wv_{dc}")
                    nc.gpsimd.dma_start(out=t, in_=cast(wv_in[dc * 128 : (dc + 1) * 128, :]))
                    wv.append(t)

                for qb in range(NQB):
                    q0 = qb * QB
                    qt_blk = qtin_pool.tile(
                        [128, OC * QB], CD, tag="qtin", bufs=2, name=f"qtin_{qb}"
                    )
                    if qb == 0:
                        for oc in range(OC):
                            nc.sync.dma_start(
                                out=qt_blk[:, oc * QB : (oc + 1) * QB],
                                in_=cast(
                                    qt_t[oc * 128 : (oc + 1) * 128, q0 : q0 + QB]
                                ),
                            )
                    else:
                        nc.sync.dma_start(
                            out=qt_blk.rearrange("p (c q) -> p c q", c=OC),
                            in_=cast(qt_t[:, q0 : q0 + QB]).rearrange(
                                "(c p) q -> p c q", p=128
                            ),
                        )
                    qts = [qt_blk[:, oc * QB : (oc + 1) * QB] for oc in range(OC)]
                    # scoresT[ks, q]: one full PSUM bank per kc (QB=512 wide),
                    # exp straight to bf16 (halves SBUF + DVE traffic).
                    expT = []
                    for kc in range(KC):
                        ps_s = pcs.tile([128, QB], F32, tag="pcs", bufs=3)
                        sb, ss = kc // (SB // 128), kc % (SB // 128)
                        for oc in range(OC):
                            nc.tensor.matmul(
                                ps_s,
                                kt[oc][sb][:, ss * 128 : (ss + 1) * 128],
                                qts[oc],
                                start=(oc == 0), stop=(oc == OC - 1),
                            )
                        e = exp_pool.tile([128, QB], BF16, tag="expT", bufs=KC + 1)
                        nc.scalar.activation(out=e, in_=ps_s, func=EXP, scale=SCALE)
                        expT.append(e)
                    # A^T[d, q] = sum_ks x[ks, d] * expT[ks, q]  (bf16 operands)
                    aT = []
                    for dc in range(DC if 'v' in phases else 0):
                        ps_a = pca.tile([128, QB], F32, tag="pca", bufs=2)
                        for kc in range(KC):
                            nc.tensor.matmul(
                                ps_a,
                                xn2[kc][:, dc * 128 : (dc + 1) * 128],
                                expT[kc],
                                start=(kc == 0), stop=(kc == KC - 1),
                            )
                        a_t = at_pool.tile([128, QB], CD, tag="aT", bufs=DC)
                        nc.vector.tensor_copy(out=a_t, in_=ps_a)
                        aT.append(a_t)
                    # row sums (over ks = partitions) via ones-matmul, then 1/x
                    recips = []
                    for qc in range(QC if 'u' in phases else 0):
                        ps_sum = pcsum.tile([128, 1], F32, tag="pcsum", bufs=1)
                        for kc in range(KC):
                            nc.tensor.matmul(
                                ps_sum,
                                expT[kc][:, qc * 128 : (qc + 1) * 128],
                                ones_b,
                                start=(kc == 0), stop=(kc == KC - 1),
                            )
                        rc = small_pool.tile([128, 1], F32, tag="recip", bufs=4 * QC)
                        nc.vector.reciprocal(out=rc, in_=ps_sum)
                        recips.append(rc)
                    # out[q, o] = A @ Wv, normalized
                    for qc in range(QC if 'o' in phases else 0):
                        for oh in range(OH):
                            ps_o = pco.tile([128, OHW], F32, tag="pco", bufs=2)
                            for dc in range(DC):
                                nc.tensor.matmul(
                                    ps_o,
                                    aT[dc][:, qc * 128 : (qc + 1) * 128],
                                    wv[dc][:, oh * OHW : (oh + 1) * OHW],
                                    start=(dc == 0), stop=(dc == DC - 1),
                                )
                            os_ = outs_pool.tile([128, OHW], F32, tag="outs", bufs=2)
                            nc.vector.tensor_scalar_mul(out=os_, in0=ps_o, scalar1=recips[qc])
                            nc.sync.dma_start(
                                out=out_d[
                                    q0 + qc * 128 : q0 + (qc + 1) * 128,
                                    oh * OHW : (oh + 1) * OHW,
                                ],
                                in_=os_,
                            )

        top.close()

    nc.compile()
    return nc


_NC_CACHE = {}


def _get_nc():
    key = "full"
    if key not in _NC_CACHE:
        _NC_CACHE[key] = build_attn()
    return _NC_CACHE[key]


def prepare_weights(w):
    """Host-side weight prep: M = Wq Wk^T (f32) and Wv, both [D, O]."""
    m = (w[0].astype(np.float64) @ w[1].T.astype(np.float64)).astype(np.float32)
    return np.ascontiguousarray(m), np.ascontiguousarray(w[2], np.float32)


def kernel(**inputs):
    """Full-input entry point: x [8, 2048, 1024], kernel [3, 1024, 1024]."""
    from concourse.bass_utils import run_bass_kernel_spmd

    x = np.ascontiguousarray(inputs["x"], dtype=np.float32)
    w = np.ascontiguousarray(inputs["kernel"], dtype=np.float32)
    B = x.shape[0]
    m, wv = prepare_weights(w)
    nc = _get_nc()
    in_maps = [{"x": x[b], "m": m, "wv": wv} for b in range(B)]
    res = run_bass_kernel_spmd(nc, in_maps, core_ids=list(range(B)))
    return np.stack([res.results[b]["out"] for b in range(B)], axis=0)


# revision 21
# speedup vs baseline: 15.3648x; 1.1604x over previous
"""Single-head attention Trainium2 kernel (batch=8 data-parallel over 8 cores).

Reference computation (per batch element b):
    Q = x @ Wq; K = x @ Wk; V = x @ Wv          (x: [S, D], W*: [D, O])
    out = softmax(Q @ K.T * SCALE) @ V          (SCALE = 1/8, hardcoded sqrt(64))

Kernel strategy (per core, one batch element), all matmuls f32r:
  Host precomputes M = Wq @ Wk^T (exact f32), so scores = (x M) x^T and the
  device never materializes Q or K separately:
  Phase A: x -> xT via PE transposes (d on partitions).
  Phase B: G'^T = (xM)^T streamed to DRAM scratch (plays the role of Q^T);
           "K^T" is just xT, copied to phase-C-lived tiles. This removes the
           entire K projection (one of the three S*D*O matmuls) versus the
           plain QKV formulation.
  Phase C (per q-block): scoresT[ks,q] = xT-chunks @ G'T-block,
           expT = exp(SCALE*scoresT) (ACT, fused scale),
           row-sums via ones-matmul -> reciprocal,
           A^T[d,q] = x-chunks.T @ expT (reassociation: attn @ x),
           out[q,o] = (A^T).T @ Wv, normalized by reciprocal on eviction.
"""
import sys
sys.path.insert(0, "/opt/trn_rl_repo")
from contextlib import ExitStack
import numpy as np
import concourse.bass as bass
import concourse.mybir as mybir
from concourse import bacc
from concourse.tile import TileContext
from concourse.masks import make_identity

F32 = mybir.dt.float32
F32R = mybir.dt.float32r
BF16 = mybir.dt.bfloat16
EXP = mybir.ActivationFunctionType.Exp
SCALE = 1.0 / 8.0


def build_attn(S=2048, D=1024, O=1024, QB=512, compute_dtype=F32R, reps=1, phases='abcuvo'):
    CD = compute_dtype
    SB = 512
    NSB = S // SB
    DC = D // 128
    OC = O // 128
    KC = S // 128
    NQB = S // QB
    QC = QB // 128
    OH = (O + 511) // 512
    OHW = min(O, 512)

    nc = bacc.Bacc("TRN2", target_bir_lowering=False, debug=False)
    x_in = nc.dram_tensor("x", [S, D], F32, kind="ExternalInput")
    m_in = nc.dram_tensor("m", [D, O], F32, kind="ExternalInput")
    wv_in = nc.dram_tensor("wv", [D, O], F32, kind="ExternalInput")
    out_d = nc.dram_tensor("out", [S, O], F32, kind="ExternalOutput")

    def cast(ap):
        return ap.bitcast(CD) if CD != F32 else ap

    with TileContext(nc) as tc:
      for _rep in range(reps):
        top = ExitStack()
        kt_pool = top.enter_context(tc.tile_pool(name="ktp", bufs=OC * NSB))
        gq_pool = top.enter_context(tc.tile_pool(name="gqp", bufs=OC * NSB))
        const_pool = top.enter_context(tc.tile_pool(name="constp", bufs=1))

        ident_f = const_pool.tile([128, 128], F32, tag="identf")
        make_identity(nc, ident_f)
        if CD != F32:
            ident = const_pool.tile([128, 128], CD, tag="identr")
            nc.vector.tensor_copy(out=ident, in_=ident_f)
        else:
            ident = ident_f
        ones_f = const_pool.tile([128, 1], F32, tag="onesf")
        nc.gpsimd.memset(ones_f, 1.0)
        ones_b = const_pool.tile([128, 1], BF16, tag="onesb")
        nc.gpsimd.memset(ones_b, 1.0)

        kt = [[None] * NSB for _ in range(OC)]
        gqt = [[None] * NSB for _ in range(OC)]

        with ExitStack() as ph_ab:
            xn_pool = ph_ab.enter_context(tc.tile_pool(name="xnp", bufs=6))
            xt_pool = ph_ab.enter_context(tc.tile_pool(name="xtp", bufs=DC * NSB))
            w_pool = ph_ab.enter_context(tc.tile_pool(name="wp", bufs=4 * DC))
            psA = ph_ab.enter_context(tc.tile_pool(name="psA", bufs=4, space="PSUM"))
            psB = ph_ab.enter_context(tc.tile_pool(name="psB", bufs=4, space="PSUM"))

            xt = [[None] * NSB for _ in range(DC)]
            xn_sb = [None] * (SB // 128)
            _mid = (NSB + 1) // 2
            _halves = [h for h in (list(range(0, _mid)), list(range(_mid, NSB))) if h]
            for half in range(len(_halves) if 'b' in phases else 0):
                sbs = _halves[half]
                for sb in sbs:
                    for ss in range(SB // 128):
                        kc = sb * (SB // 128) + ss
                        xn_t = xn_pool.tile([128, D], CD, tag="xn", bufs=6)
                        dma_eng = nc.sync if kc % 2 == 0 else nc.gpsimd
                        dma_eng.dma_start(
                            out=xn_t, in_=cast(x_in[kc * 128 : (kc + 1) * 128, :])
                        )
                        xn_sb[ss] = xn_t
                    for dc in range(DC):
                        ps = psA.tile([128, SB], CD, tag="pst", bufs=4)
                        for s2 in range(SB // 128):
                            nc.tensor.transpose(
                                ps[:, s2 * 128 : (s2 + 1) * 128],
                                xn_sb[s2][:, dc * 128 : (dc + 1) * 128],
                                ident,
                            )
                        xt[dc][sb] = xt_pool.tile(
                            [128, SB], CD, tag="xt", bufs=DC * NSB, name=f"xt_{dc}_{sb}"
                        )
                        nc.vector.tensor_copy(out=xt[dc][sb], in_=ps)
                for oc in range(OC):
                    wq_t = w_pool.tile(
                        [128, DC * 128], CD, tag="wq", bufs=3, name=f"wq_{half}_{oc}"
                    )
                    nc.sync.dma_start(
                        out=wq_t.rearrange("p (c o) -> p c o", c=DC),
                        in_=cast(m_in[:, oc * 128 : (oc + 1) * 128]).rearrange(
                            "(c p) o -> p c o", p=128
                        ),
                    )
                    wq_col = [wq_t[:, dc * 128 : (dc + 1) * 128] for dc in range(DC)]
                    for j, sb in enumerate(sbs):
                        ps_q = psB.tile([128, SB], F32, tag="psb", bufs=4)
                        for dc in range(DC):
                            nc.tensor.matmul(
                                ps_q, wq_col[dc], xt[dc][sb],
                                start=(dc == 0), stop=(dc == DC - 1),
                            )
                        # G'T stays resident in SBUF as bf16 (no DRAM trip)
                        gqt[oc][sb] = gq_pool.tile(
                            [128, SB], BF16, tag="gq", bufs=OC * NSB,
                            name=f"gq_{oc}_{sb}"
                        )
                        nc.vector.tensor_copy(out=gqt[oc][sb], in_=ps_q)
                        # "K^T" for scores is just x^T, rounded to bf16
                        if kt[oc][sb] is None:
                            kt_tile = kt_pool.tile(
                                [128, SB], BF16, tag="kt", bufs=OC * NSB,
                                name=f"kt_{oc}_{sb}"
                            )
                            nc.scalar.copy(out=kt_tile,
                                           in_=xt[oc][sb].bitcast(F32))
                            kt[oc][sb] = kt_tile

        with ExitStack() as ph_c:
          if 'c' in phases:
                xn2_pool = ph_c.enter_context(tc.tile_pool(name="xn2p", bufs=KC))
                wv_pool = ph_c.enter_context(tc.tile_pool(name="wvp", bufs=DC))
                exp_pool = ph_c.enter_context(tc.tile_pool(name="expp", bufs=KC + 1))
                at_pool = ph_c.enter_context(tc.tile_pool(name="atp", bufs=DC))
                outs_pool = ph_c.enter_context(tc.tile_pool(name="outsp", bufs=2))
                small_pool = ph_c.enter_context(tc.tile_pool(name="smallp", bufs=4 * QC))
                pcs = ph_c.enter_context(tc.tile_pool(name="pcs", bufs=3, space="PSUM"))
                pcsum = ph_c.enter_context(tc.tile_pool(name="pcsum", bufs=1, space="PSUM"))
                pca = ph_c.enter_context(tc.tile_pool(name="pca", bufs=2, space="PSUM"))
                pco = ph_c.enter_context(tc.tile_pool(name="pco", bufs=2, space="PSUM"))

                xn2 = []
                for kc in range(KC):
                    t = xn2_pool.tile([128, D], BF16, tag="xn2", bufs=KC, name=f"xn2_{kc}")
                    nc.gpsimd.dma_start(out=t, in_=x_in[kc * 128 : (kc + 1) * 128, :])
                    xn2.append(t)
                wv = []
                for dc in range(DC):
                    t = wv_pool.tile([128, O], CD, tag="wv", bufs=DC, name=f"wv_{dc}")
                    nc.gpsimd.dma_start(out=t, in_=cast(wv_in[dc * 128 : (dc + 1) * 128, :]))
                    wv.append(t)

                for qb in range(NQB):
                    q0 = qb * QB
                    qts = [gqt[oc][qb] for oc in range(OC)]
                    # scoresT[ks, q]: one full PSUM bank per kc (QB=512 wide),
                    # exp straight to bf16 (halves SBUF + DVE traffic).
                    expT = []
                    for kc in range(KC):
                        ps_s = pcs.tile([128, QB], F32, tag="pcs", bufs=3)
                        sb, ss = kc // (SB // 128), kc % (SB // 128)
                        for oc in range(OC):
                            nc.tensor.matmul(
                                ps_s,
                                kt[oc][sb][:, ss * 128 : (ss + 1) * 128],
                                qts[oc],
                                start=(oc == 0), stop=(oc == OC - 1),
                            )
                        e = exp_pool.tile([128, QB], BF16, tag="expT", bufs=KC + 1)
                        nc.scalar.activation(out=e, in_=ps_s, func=EXP, scale=SCALE)
                        expT.append(e)
                    # A^T[d, q] = sum_ks x[ks, d] * expT[ks, q]  (bf16 operands)
                    aT = []
                    for dc in range(DC if 'v' in phases else 0):
                        ps_a = pca.tile([128, QB], F32, tag="pca", bufs=2)
                        for kc in range(KC):
                            nc.tensor.matmul(
                                ps_a,
                                xn2[kc][:, dc * 128 : (dc + 1) * 128],
                                expT[kc],
                                start=(kc == 0), stop=(kc == KC - 1),
                            )
                        a_t = at_pool.tile([128, QB], CD, tag="aT", bufs=DC)
                        nc.vector.tensor_copy(out=a_t, in_=ps_a)
                        aT.append(a_t)
                    # row sums (over ks = partitions) via ones-matmul, then 1/x
                    recips = []
                    for qc in range(QC if 'u' in phases else 0):
                        ps_sum = pcsum.tile([128, 1], F32, tag="pcsum", bufs=1)
                        for kc in range(KC):
                            nc.tensor.matmul(
                                ps_sum,
                                expT[kc][:, qc * 128 : (qc + 1) * 128],
                                ones_b,
                                start=(kc == 0), stop=(kc == KC - 1),
                            )
                        rc = small_pool.tile([128, 1], F32, tag="recip", bufs=4 * QC)
                        nc.vector.reciprocal(out=rc, in_=ps_sum)
                        recips.append(rc)
                    # out[q, o] = A @ Wv, normalized
                    for qc in range(QC if 'o' in phases else 0):
                        for oh in range(OH):
                            ps_o = pco.tile([128, OHW], F32, tag="pco", bufs=2)
                            for dc in range(DC):
                                nc.tensor.matmul(
                                    ps_o,
                                    aT[dc][:, qc * 128 : (qc + 1) * 128],
                                    wv[dc][:, oh * OHW : (oh + 1) * OHW],
                                    start=(dc == 0), stop=(dc == DC - 1),
                                )
                            os_ = outs_pool.tile([128, OHW], F32, tag="outs", bufs=2)
                            nc.vector.tensor_scalar_mul(out=os_, in0=ps_o, scalar1=recips[qc])
                            nc.sync.dma_start(
                                out=out_d[
                                    q0 + qc * 128 : q0 + (qc + 1) * 128,
                                    oh * OHW : (oh + 1) * OHW,
                                ],
                                in_=os_,
                            )

        top.close()

    nc.compile()
    return nc


_NC_CACHE = {}


def _get_nc():
    key = "full"
    if key not in _NC_CACHE:
        _NC_CACHE[key] = build_attn()
    return _NC_CACHE[key]


def prepare_weights(w):
    """Host-side weight prep: M = Wq Wk^T (f32) and Wv, both [D, O]."""
    m = (w[0].astype(np.float64) @ w[1].T.astype(np.float64)).astype(np.float32)
    return np.ascontiguousarray(m), np.ascontiguousarray(w[2], np.float32)


def kernel(**inputs):
    """Full-input entry point: x [8, 2048, 1024], kernel [3, 1024, 1024]."""
    from concourse.bass_utils import run_bass_kernel_spmd

    x = np.ascontiguousarray(inputs["x"], dtype=np.float32)
    w = np.ascontiguousarray(inputs["kernel"], dtype=np.float32)
    B = x.shape[0]
    m, wv = prepare_weights(w)
    nc = _get_nc()
    in_maps = [{"x": x[b], "m": m, "wv": wv} for b in range(B)]
    res = run_bass_kernel_spmd(nc, in_maps, core_ids=list(range(B)))
    return np.stack([res.results[b]["out"] for b in range(B)], axis=0)


# revision 22
# speedup vs baseline: 16.7893x; 1.0927x over previous
"""Single-head attention Trainium2 kernel (batch=8 data-parallel over 8 cores).

Reference computation (per batch element b):
    Q = x @ Wq; K = x @ Wk; V = x @ Wv          (x: [S, D], W*: [D, O])
    out = softmax(Q @ K.T * SCALE) @ V          (SCALE = 1/8, hardcoded sqrt(64))

Kernel strategy (per core, one batch element), all matmuls f32r:
  Host precomputes M = Wq @ Wk^T (exact f32), so scores = (x M) x^T and the
  device never materializes Q or K separately:
  Phase A: x -> xT via PE transposes (d on partitions).
  Phase B: G'^T = (xM)^T streamed to DRAM scratch (plays the role of Q^T);
           "K^T" is just xT, copied to phase-C-lived tiles. This removes the
           entire K projection (one of the three S*D*O matmuls) versus the
           plain QKV formulation.
  Phase C (per q-block): scoresT[ks,q] = xT-chunks @ G'T-block,
           expT = exp(SCALE*scoresT) (ACT, fused scale),
           row-sums via ones-matmul -> reciprocal,
           A^T[d,q] = x-chunks.T @ expT (reassociation: attn @ x),
           out[q,o] = (A^T).T @ Wv, normalized by reciprocal on eviction.
"""
import sys
sys.path.insert(0, "/opt/trn_rl_repo")
from contextlib import ExitStack
import numpy as np
import concourse.bass as bass
import concourse.mybir as mybir
from concourse import bacc
from concourse.tile import TileContext
from concourse.masks import make_identity

F32 = mybir.dt.float32
F32R = mybir.dt.float32r
BF16 = mybir.dt.bfloat16
EXP = mybir.ActivationFunctionType.Exp
SCALE = 1.0 / 8.0


def build_attn(S=2048, D=1024, O=1024, QB=512, compute_dtype=F32R, reps=1, phases='abcuvo'):
    CD = compute_dtype
    SB = 512
    NSB = S // SB
    DC = D // 128
    OC = O // 128
    KC = S // 128
    NQB = S // QB
    QC = QB // 128
    OH = (O + 511) // 512
    OHW = min(O, 512)

    nc = bacc.Bacc("TRN2", target_bir_lowering=False, debug=False)
    x_in = nc.dram_tensor("x", [S, D], F32, kind="ExternalInput")
    m_in = nc.dram_tensor("m", [D, O], F32, kind="ExternalInput")
    wv_in = nc.dram_tensor("wv", [D, O], F32, kind="ExternalInput")
    out_d = nc.dram_tensor("out", [S, O], F32, kind="ExternalOutput")

    def cast(ap):
        return ap.bitcast(CD) if CD != F32 else ap

    with TileContext(nc) as tc:
      for _rep in range(reps):
        top = ExitStack()
        kt_pool = top.enter_context(tc.tile_pool(name="ktp", bufs=OC * NSB))
        gq_pool = top.enter_context(tc.tile_pool(name="gqp", bufs=OC * NSB))
        const_pool = top.enter_context(tc.tile_pool(name="constp", bufs=1))

        ident_f = const_pool.tile([128, 128], F32, tag="identf")
        make_identity(nc, ident_f)
        if CD != F32:
            ident = const_pool.tile([128, 128], CD, tag="identr")
            nc.vector.tensor_copy(out=ident, in_=ident_f)
        else:
            ident = ident_f
        ones_f = const_pool.tile([128, 1], F32, tag="onesf")
        nc.gpsimd.memset(ones_f, 1.0)
        ones_b = const_pool.tile([128, 1], BF16, tag="onesb")
        nc.gpsimd.memset(ones_b, 1.0)

        kt = [[None] * NSB for _ in range(OC)]
        gqt = [[None] * NSB for _ in range(OC)]

        with ExitStack() as ph_ab:
            xn_pool = ph_ab.enter_context(tc.tile_pool(name="xnp", bufs=6))
            xt_pool = ph_ab.enter_context(tc.tile_pool(name="xtp", bufs=DC * NSB))
            w_pool = ph_ab.enter_context(tc.tile_pool(name="wp", bufs=4 * DC))
            psA = ph_ab.enter_context(tc.tile_pool(name="psA", bufs=4, space="PSUM"))
            psB = ph_ab.enter_context(tc.tile_pool(name="psB", bufs=4, space="PSUM"))

            xt = [[None] * NSB for _ in range(DC)]
            xn_sb = [None] * (SB // 128)
            _mid = (NSB + 1) // 2
            _halves = [h for h in (list(range(0, _mid)), list(range(_mid, NSB))) if h]
            for half in range(len(_halves) if 'b' in phases else 0):
                sbs = _halves[half]
                for sb in sbs:
                    for ss in range(SB // 128):
                        kc = sb * (SB // 128) + ss
                        xn_t = xn_pool.tile([128, D], CD, tag="xn", bufs=6)
                        dma_eng = nc.sync if kc % 2 == 0 else nc.gpsimd
                        dma_eng.dma_start(
                            out=xn_t, in_=cast(x_in[kc * 128 : (kc + 1) * 128, :])
                        )
                        xn_sb[ss] = xn_t
                    for dc in range(DC):
                        ps = psA.tile([128, SB], CD, tag="pst", bufs=4)
                        for s2 in range(SB // 128):
                            nc.tensor.transpose(
                                ps[:, s2 * 128 : (s2 + 1) * 128],
                                xn_sb[s2][:, dc * 128 : (dc + 1) * 128],
                                ident,
                            )
                        xt[dc][sb] = xt_pool.tile(
                            [128, SB], CD, tag="xt", bufs=DC * NSB, name=f"xt_{dc}_{sb}"
                        )
                        nc.vector.tensor_copy(out=xt[dc][sb], in_=ps)
                for oc in range(OC):
                    wq_t = w_pool.tile(
                        [128, DC * 128], CD, tag="wq", bufs=3, name=f"wq_{half}_{oc}"
                    )
                    nc.sync.dma_start(
                        out=wq_t.rearrange("p (c o) -> p c o", c=DC),
                        in_=cast(m_in[:, oc * 128 : (oc + 1) * 128]).rearrange(
                            "(c p) o -> p c o", p=128
                        ),
                    )
                    wq_col = [wq_t[:, dc * 128 : (dc + 1) * 128] for dc in range(DC)]
                    ps_qs = [psB.tile([128, SB], F32, tag="psb", bufs=4,
                                      name=f"psb_{half}_{oc}_{j}")
                             for j in range(len(sbs))]
                    for dc in range(DC):
                        for j, sb in enumerate(sbs):
                            nc.tensor.matmul(
                                ps_qs[j], wq_col[dc], xt[dc][sb],
                                start=(dc == 0), stop=(dc == DC - 1),
                            )
                    for j, sb in enumerate(sbs):
                        # G'T stays resident in SBUF as bf16 (no DRAM trip)
                        gqt[oc][sb] = gq_pool.tile(
                            [128, SB], BF16, tag="gq", bufs=OC * NSB,
                            name=f"gq_{oc}_{sb}"
                        )
                        nc.vector.tensor_copy(out=gqt[oc][sb], in_=ps_qs[j])
                        # "K^T" for scores is just x^T, rounded to bf16
                        if kt[oc][sb] is None:
                            kt_tile = kt_pool.tile(
                                [128, SB], BF16, tag="kt", bufs=OC * NSB,
                                name=f"kt_{oc}_{sb}"
                            )
                            nc.scalar.copy(out=kt_tile,
                                           in_=xt[oc][sb].bitcast(F32))
                            kt[oc][sb] = kt_tile

        with ExitStack() as ph_c:
          if 'c' in phases:
                xn2_pool = ph_c.enter_context(tc.tile_pool(name="xn2p", bufs=KC))
                wv_pool = ph_c.enter_context(tc.tile_pool(name="wvp", bufs=DC))
                exp_pool = ph_c.enter_context(tc.tile_pool(name="expp", bufs=KC + 1))
                at_pool = ph_c.enter_context(tc.tile_pool(name="atp", bufs=DC))
                outs_pool = ph_c.enter_context(tc.tile_pool(name="outsp", bufs=2))
                small_pool = ph_c.enter_context(tc.tile_pool(name="smallp", bufs=4 * QC))
                pcs = ph_c.enter_context(tc.tile_pool(name="pcs", bufs=3, space="PSUM"))
                pcsum = ph_c.enter_context(tc.tile_pool(name="pcsum", bufs=1, space="PSUM"))
                pca = ph_c.enter_context(tc.tile_pool(name="pca", bufs=2, space="PSUM"))
                pco = ph_c.enter_context(tc.tile_pool(name="pco", bufs=2, space="PSUM"))

                xn2 = []
                for kc in range(KC):
                    t = xn2_pool.tile([128, D], BF16, tag="xn2", bufs=KC, name=f"xn2_{kc}")
                    nc.gpsimd.dma_start(out=t, in_=x_in[kc * 128 : (kc + 1) * 128, :])
                    xn2.append(t)
                wv = []
                for dc in range(DC):
                    t = wv_pool.tile([128, O], CD, tag="wv", bufs=DC, name=f"wv_{dc}")
                    nc.gpsimd.dma_start(out=t, in_=cast(wv_in[dc * 128 : (dc + 1) * 128, :]))
                    wv.append(t)

                for qb in range(NQB):
                    q0 = qb * QB
                    qts = [gqt[oc][qb] for oc in range(OC)]
                    # scoresT[ks, q]: one full PSUM bank per kc (QB=512 wide),
                    # exp straight to bf16 (halves SBUF + DVE traffic).
                    expT = []
                    for kc in range(KC):
                        ps_s = pcs.tile([128, QB], F32, tag="pcs", bufs=3)
                        sb, ss = kc // (SB // 128), kc % (SB // 128)
                        for oc in range(OC):
                            nc.tensor.matmul(
                                ps_s,
                                kt[oc][sb][:, ss * 128 : (ss + 1) * 128],
                                qts[oc],
                                start=(oc == 0), stop=(oc == OC - 1),
                            )
                        e = exp_pool.tile([128, QB], BF16, tag="expT", bufs=KC + 1)
                        nc.scalar.activation(out=e, in_=ps_s, func=EXP, scale=SCALE)
                        expT.append(e)
                    # A^T[d, q] = sum_ks x[ks, d] * expT[ks, q]  (bf16 operands)
                    aT = []
                    for dc in range(DC if 'v' in phases else 0):
                        ps_a = pca.tile([128, QB], F32, tag="pca", bufs=2)
                        for kc in range(KC):
                            nc.tensor.matmul(
                                ps_a,
                                xn2[kc][:, dc * 128 : (dc + 1) * 128],
                                expT[kc],
                                start=(kc == 0), stop=(kc == KC - 1),
                            )
                        a_t = at_pool.tile([128, QB], CD, tag="aT", bufs=DC)
                        nc.vector.tensor_copy(out=a_t, in_=ps_a)
                        aT.append(a_t)
                    # row sums (over ks = partitions) via ones-matmul, then 1/x
                    recips = []
                    for qc in range(QC if 'u' in phases else 0):
                        ps_sum = pcsum.tile([128, 1], F32, tag="pcsum", bufs=1)
                        for kc in range(KC):
                            nc.tensor.matmul(
                                ps_sum,
                                expT[kc][:, qc * 128 : (qc + 1) * 128],
                                ones_b,
                                start=(kc == 0), stop=(kc == KC - 1),
                            )
                        rc = small_pool.tile([128, 1], F32, tag="recip", bufs=4 * QC)
                        nc.vector.reciprocal(out=rc, in_=ps_sum)
                        recips.append(rc)
                    # out[q, o] = A @ Wv, normalized
                    for qc in range(QC if 'o' in phases else 0):
                        ps_os = [pco.tile([128, OHW], F32, tag="pco", bufs=2,
                                          name=f"pco_{qb}_{qc}_{oh}")
                                 for oh in range(OH)]
                        for dc in range(DC):
                            st = aT[dc][:, qc * 128 : (qc + 1) * 128]
                            for oh in range(OH):
                                nc.tensor.matmul(
                                    ps_os[oh], st,
                                    wv[dc][:, oh * OHW : (oh + 1) * OHW],
                                    start=(dc == 0), stop=(dc == DC - 1),
                                )
                        for oh in range(OH):
                            os_ = outs_pool.tile([128, OHW], F32, tag="outs", bufs=2)
                            nc.vector.tensor_scalar_mul(out=os_, in0=ps_os[oh], scalar1=recips[qc])
                            nc.sync.dma_start(
                                out=out_d[
                                    q0 + qc * 128 : q0 + (qc + 1) * 128,
                                    oh * OHW : (oh + 1) * OHW,
                                ],
                                in_=os_,
                            )

        top.close()

    nc.compile()
    return nc


_NC_CACHE = {}


def _get_nc():
    key = "full"
    if key not in _NC_CACHE:
        _NC_CACHE[key] = build_attn()
    return _NC_CACHE[key]


def prepare_weights(w):
    """Host-side weight prep: M = Wq Wk^T (f32) and Wv, both [D, O]."""
    m = (w[0].astype(np.float64) @ w[1].T.astype(np.float64)).astype(np.float32)
    return np.ascontiguousarray(m), np.ascontiguousarray(w[2], np.float32)


def kernel(**inputs):
    """Full-input entry point: x [8, 2048, 1024], kernel [3, 1024, 1024]."""
    from concourse.bass_utils import run_bass_kernel_spmd

    x = np.ascontiguousarray(inputs["x"], dtype=np.float32)
    w = np.ascontiguousarray(inputs["kernel"], dtype=np.float32)
    B = x.shape[0]
    m, wv = prepare_weights(w)
    nc = _get_nc()
    in_maps = [{"x": x[b], "m": m, "wv": wv} for b in range(B)]
    res = run_bass_kernel_spmd(nc, in_maps, core_ids=list(range(B)))
    return np.stack([res.results[b]["out"] for b in range(B)], axis=0)
